# revision 80
# baseline (speedup 1.0000x reference)
# Trainium2 Bass kernel for nn_CFTAuxHead (bilinear 4x resize + bbox
# rasterization + MSE loss), data-parallel over batch across 8 NeuronCores.
#
# Math summary (per sample):
#   feat_up = A^T @ feat @ A  (A = exact 160->640 bilinear weight matrix,
#                              all weights exactly representable in bf16)
#   heatmap = last-writer-wins paint of 128 axis-aligned rects (value z_n)
#   loss    = mean((feat_up - heatmap)^2) over all pixels
#
# Rasterization on device: 2 "paint" matmuls over box interval-indicator
# matrices U[n, row], V[n, col], single-group exponent encoding:
#   C  = eps + sum_n 2^(n'-64)        (n' = n+1; later boxes dominate)
#   CA = sum_n z_n * 2^(n'-65)
# Decode per pixel (exact for depth<=1, ~1.6e-4 loss error overall):
#   rE = bitcast((bitcast(C) & 0xFF800000) ^ 0x7F800000)   # = 2/E exactly
#   Z  = CA * rE  (bf16; uncovered pixels give CA=0 -> Z=0, eps keeps rE
#                  finite so no NaN). Each DVE op reads one PSUM tensor.
# The (feat_up - Z) subtraction runs on the PE via a -I matmul accumulate
# into the resize PSUM tile; Square+row-accumulate on the Act engine.

import numpy as np

B, C_IN, H, W = 32, 1, 160, 160
UP = 4
HO, WO = H * UP, W * UP
NBOX = 128
NCORES = 8
SPC = B // NCORES  # samples per core
NPIX = float(B * HO * WO)

MASK_I = -8388608         # 0xFF800000 as signed int32 (sign+exponent mask)
XOR_I = 0x7F800000        # flips all 8 exponent bits: e -> 255-e
EPS_C = float(2.0 ** -70)  # virtual empty box: keeps rE finite when C=0

_CACHE = {}


def _resize_matrix():
    """Exact bilinear (half-pixel centers, edge-clamped) 160->640 matrix,
    matching jax.image.resize(method='bilinear') for upsampling."""
    n_in, n_out = H, HO
    scale = n_out / n_in
    x = (np.arange(n_out, dtype=np.float64) + 0.5) / scale - 0.5
    k = np.arange(n_in, dtype=np.float64)
    w = np.maximum(0.0, 1.0 - np.abs(x[None, :] - k[:, None]))  # [in, out]
    w = w / w.sum(axis=0, keepdims=True)
    return w.astype(np.float32)


def _const_vec():
    """[128, 2]: col0 = 2^(n'-64) paint weights, col1 = 2^(n'-65)."""
    npr = np.arange(1, NBOX + 1, dtype=np.float64)
    ws = 2.0 ** (npr - 64)
    return np.stack([ws, ws * 0.5], axis=1).astype(np.float32)


def _build(krep=1):
    import os
    import concourse.bacc as bacc
    import concourse.mybir as mybir
    from concourse.tile import TileContext

    fp32 = mybir.dt.float32
    bf16 = mybir.dt.bfloat16
    fp16 = mybir.dt.float16
    i32 = mybir.dt.int32
    Alu = mybir.AluOpType
    AF = mybir.ActivationFunctionType

    nc = bacc.Bacc("TRN2", target_bir_lowering=False, debug=False,
                   enable_asserts=False, num_devices=NCORES)
    feat_d = nc.dram_tensor("feat", [SPC, H, W], bf16, kind="ExternalInput")
    box_d = nc.dram_tensor("boxes", [SPC, NBOX, 5], fp32, kind="ExternalInput")
    amat_d = nc.dram_tensor("amat", [H, HO], bf16, kind="ExternalInput")
    cvec_d = nc.dram_tensor("cvec", [NBOX, 2], fp32, kind="ExternalInput")
    out_d = nc.dram_tensor("out", [NBOX, 1], fp32, kind="ExternalOutput")

    NPIECE = krep * SPC * 10
    CB = (slice(0, 320), slice(320, 640))

    with TileContext(nc, num_cores=NCORES) as tc:
        with tc.tile_pool(name="const", bufs=1) as cpool, \
             tc.tile_pool(name="bxp", bufs=1) as bpool, \
             tc.tile_pool(name="samp", bufs=2) as spool, \
             tc.tile_pool(name="gbuf", bufs=2) as gpool, \
             tc.tile_pool(name="dec", bufs=4) as dpool, \
             tc.tile_pool(name="pnt", bufs=int(os.environ.get("KV_CB", "3")),
                          space="PSUM") as ppool, \
             tc.tile_pool(name="pnt2", bufs=int(os.environ.get("KV_CAB", "3")),
                          space="PSUM") as qpool, \
             tc.tile_pool(name="acc", bufs=int(os.environ.get("KV_TB", "2")),
                          space="PSUM") as tpool:

            # ---- DMAs + iotas first (iotas precede Pool-issued A DMAs)
            bx4 = cpool.tile([128, 20], fp32, tag="bx4")
            nc.sync.dma_start(bx4[:], box_d.ap().transpose([1, 0, 2]))
            F0all = cpool.tile([128, SPC * W], bf16, tag="F0all")
            F1all = cpool.tile([32, SPC * W], bf16, tag="F1all")
            nc.sync.dma_start(F0all[:],
                              feat_d.ap()[:, 0:128, :].transpose([1, 0, 2]))
            A1 = cpool.tile([32, HO], bf16, tag="A1")
            nc.sync.dma_start(A1[:], amat_d.ap()[128:160, :])
            nc.sync.dma_start(F1all[:],
                              feat_d.ap()[:, 128:160, :].transpose([1, 0, 2]))
            cv = cpool.tile([128, 2], fp32, tag="cv")
            nc.sync.dma_start(cv[:], cvec_d.ap())
            ws_p = cv[:, 0:1]
            wh_p = cv[:, 1:2]

            iota_i = cpool.tile([128, HO], i32, tag="ioti")
            nc.gpsimd.iota(iota_i[:], pattern=[[1, HO]], base=0,
                           channel_multiplier=0)
            ci = cpool.tile([128, 128], i32, tag="ci")
            nc.gpsimd.iota(ci[:], pattern=[[1, 128]], base=0,
                           channel_multiplier=0)
            pidx = cpool.tile([128, 1], i32, tag="pidx")
            nc.gpsimd.iota(pidx[:], pattern=[[1, 1]], base=0,
                           channel_multiplier=1)
            A0 = cpool.tile([128, HO], bf16, tag="A0")
            nc.gpsimd.dma_start(A0[:], amat_d.ap()[0:128, :])

            ones1 = cpool.tile([1, 128], bf16, tag="ones1")
            nc.gpsimd.memset(ones1[:], 1.0)
            epsrow = cpool.tile([1, HO], bf16, tag="epsrow")
            nc.gpsimd.memset(epsrow[:], EPS_C)
            iota_h = cpool.tile([128, HO], fp16, tag="ioth")
            nc.vector.tensor_copy(iota_h[:], iota_i[:])

            accbuf = cpool.tile([128, NPIECE], fp32, tag="acc")

            xall = bx4[:, 0:20:5]
            yall = bx4[:, 1:20:5]
            zall = bx4[:, 2:20:5]
            wall = bx4[:, 3:20:5]
            lall = bx4[:, 4:20:5]

            # ---- batched box prep ([128,4] ops) ----
            def floorb(src_ap, tagp, eng, pre_half=False):
                """floor(x) (or floor(x/2)) for 0 <= x < 2^23."""
                if pre_half:
                    h = bpool.tile([128, 4], fp32, tag=tagp + "_h")
                    eng.tensor_scalar(h[:], src_ap, 0.5, None, Alu.mult)
                    src_ap = h[:]
                ti = bpool.tile([128, 4], i32, tag=tagp + "_i")
                eng.tensor_copy(ti[:], src_ap)
                tf = bpool.tile([128, 4], fp32, tag=tagp + "_f")
                eng.tensor_copy(tf[:], ti[:])
                m = bpool.tile([128, 4], fp32, tag=tagp + "_m")
                eng.tensor_tensor(m[:], tf[:], src_ap, Alu.is_gt)
                fl = bpool.tile([128, 4], fp32, tag=tagp + "_o")
                eng.tensor_tensor(fl[:], tf[:], m[:], Alu.subtract)
                return fl

            cx = floorb(xall, "cx", nc.vector)
            cy = floorb(yall, "cy", nc.vector)
            hw0 = floorb(wall, "hw", nc.vector, pre_half=True)
            hl0 = floorb(lall, "hl", nc.vector, pre_half=True)
            hwv = bpool.tile([128, 4], fp32, tag="hwv")
            nc.vector.tensor_scalar(hwv[:], hw0[:], 3.0, None, Alu.max)
            hlv = bpool.tile([128, 4], fp32, tag="hlv")
            nc.vector.tensor_scalar(hlv[:], hl0[:], 3.0, None, Alu.max)

            xmin = bpool.tile([128, 4], fp32, tag="xmin")
            nc.vector.tensor_tensor(xmin[:], cx[:], hwv[:], Alu.subtract)
            xmax = bpool.tile([128, 4], fp32, tag="xmax")
            nc.vector.scalar_tensor_tensor(xmax[:], cx[:], 1.0, hwv[:],
                                           Alu.add, Alu.add)
            ymin = bpool.tile([128, 4], fp32, tag="ymin")
            nc.vector.tensor_tensor(ymin[:], cy[:], hlv[:], Alu.subtract)
            ymax = bpool.tile([128, 4], fp32, tag="ymax")
            nc.vector.scalar_tensor_tensor(ymax[:], cy[:], 1.0, hlv[:],
                                           Alu.add, Alu.add)

            # validity (w > 0 and l > 0) folded into xmax
            vv = bpool.tile([128, 4], fp32, tag="vv")
            nc.vector.scalar_tensor_tensor(vv[:], wall, 0.0, lall,
                                           Alu.is_gt, Alu.logical_and)
            xmaxe = bpool.tile([128, 4], fp32, tag="xmaxe")
            nc.vector.tensor_tensor(xmaxe[:], xmax[:], vv[:], Alu.mult)

            # per-box CA weights z * 2^(n'-65)
            wa4 = bpool.tile([128, 4], fp32, tag="wa4")
            nc.vector.tensor_scalar(wa4[:], zall, wh_p, None, Alu.mult)

            pending = []
            prev = None
            PEND_LAG = int(os.environ.get("KV_LAG", "1"))

            def flush_pending(lag=0):
                while len(pending) > lag:
                    fn = pending.pop(0)
                    fn()

            def prep_uv_dve(s):
                """U/V indicators, DVE form (scalar_tensor_tensor legal)."""
                xmn = xmin[:, s:s + 1]
                xmx = xmaxe[:, s:s + 1]
                ymn = ymin[:, s:s + 1]
                ymx = ymax[:, s:s + 1]
                wa = wa4[:, s:s + 1]
                eng = nc.vector

                tU = spool.tile([128, HO], fp16, tag=f"tU{s}",
                                name=f"tU{s}")
                eng.tensor_scalar(tU[:], iota_h[:], xmx, None, Alu.is_lt)
                U = spool.tile([128, HO], bf16, tag=f"U{s}", name=f"U{s}")
                eng.scalar_tensor_tensor(U[:], iota_h[:], xmn,
                                         tU[:], Alu.is_ge, Alu.mult)
                tVs = spool.tile([128, HO], bf16, tag=f"tVs{s}",
                                 name=f"tVs{s}")
                eng.tensor_scalar(tVs[:], iota_h[:], ymx, ws_p,
                                  Alu.is_lt, Alu.mult)
                V_s = spool.tile([128, HO], bf16, tag=f"Vs{s}",
                                 name=f"Vs{s}")
                eng.scalar_tensor_tensor(V_s[:], iota_h[:], ymn,
                                         tVs[:], Alu.is_ge, Alu.mult)
                tVa = spool.tile([128, HO], bf16, tag=f"tVa{s}",
                                 name=f"tVa{s}")
                eng.tensor_scalar(tVa[:], iota_h[:], ymx, wa,
                                  Alu.is_lt, Alu.mult)
                V_a = spool.tile([128, HO], bf16, tag=f"Va{s}",
                                 name=f"Va{s}")
                eng.scalar_tensor_tensor(V_a[:], iota_h[:], ymn,
                                         tVa[:], Alu.is_ge, Alu.mult)
                return U, V_s, V_a

            def prep_uv_pool(s):
                """U/V indicators, Pool form (tensor_scalar + TT mult)."""
                xmn = xmin[:, s:s + 1]
                xmx = xmaxe[:, s:s + 1]
                ymn = ymin[:, s:s + 1]
                ymx = ymax[:, s:s + 1]
                wa = wa4[:, s:s + 1]
                eng = nc.gpsimd

                ax = spool.tile([128, HO], fp16, tag=f"tU{s}",
                                name=f"ax{s}")
                eng.tensor_scalar(ax[:], iota_h[:], xmn, None, Alu.is_ge)
                bx = spool.tile([128, HO], fp16, tag=f"tUb{s}",
                                name=f"bx{s}")
                eng.tensor_scalar(bx[:], iota_h[:], xmx, None, Alu.is_lt)
                U = spool.tile([128, HO], bf16, tag=f"U{s}", name=f"U{s}")
                eng.tensor_tensor(U[:], ax[:], bx[:], Alu.mult)
                ay = spool.tile([128, HO], fp16, tag=f"tVs{s}",
                                name=f"ay{s}")
                eng.tensor_scalar(ay[:], iota_h[:], ymn, None, Alu.is_ge)
                bs = spool.tile([128, HO], bf16, tag=f"tVb{s}",
                                name=f"bs{s}")
                eng.tensor_scalar(bs[:], iota_h[:], ymx, ws_p,
                                  Alu.is_lt, Alu.mult)
                V_s = spool.tile([128, HO], bf16, tag=f"Vs{s}",
                                 name=f"Vs{s}")
                eng.tensor_tensor(V_s[:], ay[:], bs[:], Alu.mult)
                ba = spool.tile([128, HO], bf16, tag=f"tVa{s}",
                                name=f"ba{s}")
                eng.tensor_scalar(ba[:], iota_h[:], ymx, wa,
                                  Alu.is_lt, Alu.mult)
                V_a = spool.tile([128, HO], bf16, tag=f"Va{s}",
                                 name=f"Va{s}")
                eng.tensor_tensor(V_a[:], ay[:], ba[:], Alu.mult)
                return U, V_s, V_a

            uv = {0: prep_uv_dve(0)}

            # -identity [128,128] bf16 for the PE-side (feat_up - Z) step
            cif = cpool.tile([128, 128], fp32, tag="cif")
            nc.gpsimd.tensor_copy(cif[:], ci[:])
            pidxf = cpool.tile([128, 1], fp32, tag="pidxf")
            nc.gpsimd.tensor_copy(pidxf[:], pidx[:])
            negI = cpool.tile([128, 128], bf16, tag="negI")
            nc.gpsimd.tensor_scalar(negI[:], cif[:], pidxf[:, 0:1], -1.0,
                                    Alu.is_equal, Alu.mult)

            for s in range(1, SPC):
                uv[s] = prep_uv_pool(s)

            def step1_g(s):
                """Resize step 1: G = F^T A (bf16) for sample s."""
                F0 = F0all[:, s * W:(s + 1) * W]
                F1 = F1all[:, s * W:(s + 1) * W]
                G0 = gpool.tile([128, HO], bf16, tag="G0")
                G1 = gpool.tile([32, HO], bf16, tag="G1")
                for gi, cb in enumerate(CB):
                    Gp = ppool.tile([128, 320], fp32, tag="C",
                                    name=f"Gp{gi}")
                    nc.tensor.matmul(Gp[:], F0[:, 0:128], A0[:, cb],
                                     start=True, stop=False)
                    nc.tensor.matmul(Gp[:], F1[:, 0:128], A1[:, cb],
                                     start=False, stop=True)
                    Gq = qpool.tile([128, 320], fp32, tag="CA2",
                                    name=f"Gq{gi}")
                    nc.tensor.matmul(Gq[0:32, :], F0[:, 128:160],
                                     A0[:, cb], start=True, stop=False)
                    nc.tensor.matmul(Gq[0:32, :], F1[:, 128:160],
                                     A1[:, cb], start=False, stop=True)
                    nc.scalar.copy(G0[:, cb], Gp[:])
                    nc.scalar.copy(G1[:, cb], Gq[0:32, :])
                return G0, G1

            sample_ids = [(rep, s) for rep in range(krep)
                          for s in range(SPC)]
            state = step1_g(sample_ids[0][1])
            for j, (rep, s) in enumerate(sample_ids):
                G0, G1 = state
                U, V_s, V_a = uv[s]
                for m in range(5):
                    ms = slice(m * 128, (m + 1) * 128)
                    for ic, cb in enumerate(CB):
                        idx = j * 10 + m * 2 + ic

                        # paints (decode chain is the long pole)
                        Cp = ppool.tile([128, 320], fp32, tag="C")
                        CAp = qpool.tile([128, 320], fp32, tag="CA2")
                        nc.tensor.matmul(Cp[:], U[:, ms], V_s[:, cb],
                                         start=True, stop=False)
                        nc.tensor.matmul(Cp[:], ones1[:], epsrow[:, cb],
                                         start=False, stop=True)
                        nc.tensor.matmul(CAp[:], U[:, ms], V_a[:, cb],
                                         start=True, stop=True)

                        # resize step 2 (group closed by -Z later)
                        T4 = tpool.tile([128, 320], fp32, tag="T4")
                        nc.tensor.matmul(T4[:], G0[:, ms], A0[:, cb],
                                         start=True, stop=False,
                                         skip_group_check=True)
                        nc.tensor.matmul(T4[:], G1[:, ms], A1[:, cb],
                                         start=False, stop=False,
                                         skip_group_check=True)

                        if (m, ic) == (3, 0) and j + 1 < len(sample_ids):
                            next_state = step1_g(sample_ids[j + 1][1])

                        flush_pending(PEND_LAG)

                        # decode, software-pipelined within DVE: this
                        # piece's rE issues before the previous piece's
                        # Z-multiply so rE fills CA-paint stall bubbles
                        rE = dpool.tile([128, 320], fp32, tag="rE")
                        nc.vector.tensor_scalar(rE[:].bitcast(i32),
                                                Cp[:].bitcast(i32),
                                                MASK_I, XOR_I,
                                                Alu.bitwise_and,
                                                Alu.bitwise_xor)

                        if prev is not None:
                            pCA, prE, pT4, pidx_ = prev
                            Zb = dpool.tile([128, 320], bf16, tag="Zb")
                            nc.vector.tensor_tensor(Zb[:], pCA[:], prE[:],
                                                    Alu.mult)

                            def mk_tail(T4=pT4, Zb=Zb, idx=pidx_):
                                def tail():
                                    nc.tensor.matmul(
                                        T4[:], negI[:], Zb[:],
                                        start=False, stop=True,
                                        skip_group_check=True)
                                    dso = dpool.tile([128, 320], bf16,
                                                     tag="dso")
                                    nc.scalar.activation(
                                        dso[:], T4[:], AF.Square,
                                        accum_out=accbuf[:, idx:idx + 1])
                                return tail

                            pending.append(mk_tail())
                        prev = (CAp, rE, T4, idx)

                if j + 1 < len(sample_ids):
                    state = next_state

            flush_pending()
            pCA, prE, pT4, pidx_ = prev
            ZbL = dpool.tile([128, 320], bf16, tag="Zb", name="ZbL")
            nc.vector.tensor_tensor(ZbL[:], pCA[:], prE[:], Alu.mult)
            nc.tensor.matmul(pT4[:], negI[:], ZbL[:],
                             start=False, stop=True, skip_group_check=True)
            dsoL = dpool.tile([128, 320], bf16, tag="dso", name="dsoL")
            nc.scalar.activation(dsoL[:], pT4[:], AF.Square,
                                 accum_out=accbuf[:, pidx_:pidx_ + 1])

            # ---- final reduction: per-partition partial sums to host ----
            tot = cpool.tile([128, 1], fp32, tag="tot")
            nc.vector.tensor_reduce(tot[:], accbuf[:, 0:NPIECE],
                                    mybir.AxisListType.X, Alu.add)
            nc.sync.dma_start(out_d.ap(), tot[:])

    nc.compile()
    return nc


def _get_nc(krep=1):
    key = ("nc", krep)
    if key not in _CACHE:
        _CACHE[key] = _build(krep)
    return _CACHE[key]


def run_cores(feat, gt_bboxes, krep=1):
    """Run the SPMD kernel; returns list of per-core sum-of-squared-diffs."""
    import ml_dtypes
    from concourse.bass_utils import run_bass_kernel_spmd
    nc = _get_nc(krep)
    bfdt = ml_dtypes.bfloat16
    amat = _resize_matrix().astype(bfdt)
    cvec = _const_vec()
    feat = np.asarray(feat, dtype=np.float32).astype(bfdt)
    gt = np.ascontiguousarray(np.asarray(gt_bboxes, dtype=np.float32))
    in_maps = []
    for i in range(NCORES):
        sl = slice(i * SPC, (i + 1) * SPC)
        in_maps.append({
            "feat": np.ascontiguousarray(feat[sl, 0]),
            "boxes": np.ascontiguousarray(gt[sl]),
            "amat": amat,
            "cvec": cvec,
        })
    res = run_bass_kernel_spmd(nc, in_maps, core_ids=list(range(NCORES)))
    return [float(np.sum(np.asarray(res.results[i]["out"], np.float64)))
            for i in range(NCORES)]


def kernel(feat, gt_bboxes):
    parts = run_cores(feat, gt_bboxes, krep=1)
    total = float(np.sum(np.asarray(parts, dtype=np.float64)))
    return np.asarray(np.float32(total / NPIX))
